# revision 22
# baseline (speedup 1.0000x reference)
"""Trainium2 Bass kernel for nn_DressedQuantumNet.

Math reformulation (exact, up to float rounding):
  pre_out = x @ pre_w.T + pre_b                  # [B,4]
  theta_w = (pi/4)*tanh(pre_out_w) + pi/4        # in (0, pi/2)
  v_w     = [cos theta_w, sin theta_w]           # per-qubit state (positive)
  psi     = v_0 (x) v_1 (x) v_2 (x) v_3          # [B,16] product state
  phi     = M @ psi        # M = fixed 16x16 matrix of the CNOT/RY circuit
  out     = (phi*phi)^T P + post_b  # P[i,c] = sum_w post_w[c,w] * z_w(i)

Device strategy (pure data parallel over 8 cores, 8192 samples each):
  - x bf16, loaded transposed via the DMA xbar on the sync queue; the 16
    group transposes are the critical path, so the sync queue carries
    only 3 packed const loads followed by the 16 transposes.
  - pre-matmul is PE col-tiled: the n groups of a unit go to col-strips
    32j of the PE array (tile_position), so their 512-col matmuls run
    concurrently and the psum output is a dense [32n, 512] tile.
  - units are tapered [4,4,4,2,2] groups so the last unit's dependent
    chain (the kernel tail after the final transpose) is short.
  - tanh is one [32n,512] activation per unit; the [feature, sample] ->
    [sample, feature] flip is 4 PE transposes per unit (bf16 -> psum);
    no SBUF->SBUF xbar transposes, no memset.
  - trig on ScalarE reads the transposed psum directly (2x Sin with
    scale/bias folding cos); psi built with 3 broadcast-AP vector mults.
  - quantum circuit: PE transpose of psi -> [(tile,comp), sample], then
    block-diagonal M (16x16 x8) and P (16x10 x8) matmuls in float32r
    (full-rate rows, ~fp32 accuracy).
  - per-unit output stores on the scalar queue overlap the x stream.
  - every engine queue is pinned to emission order (sync=False deps):
    queues execute in-order at runtime, and the Tile scheduler's DMA
    model otherwise reorders them badly around the long transposes.
"""

import os
import sys

for _p in ("/opt/trn_rl_repo",):
    if os.path.isdir(_p) and _p not in sys.path:
        sys.path.insert(0, _p)

import math
import numpy as np
import ml_dtypes
from contextlib import ExitStack

import concourse.bass as bass
import concourse.bacc as bacc
import concourse.mybir as mybir
from concourse.tile import TileContext, add_dep_helper
from concourse.bass_utils import run_bass_kernel_spmd

F32 = mybir.dt.float32
F32R = mybir.dt.float32r
BF16 = mybir.dt.bfloat16
AF = mybir.ActivationFunctionType
PI4 = math.pi / 4.0

PIN_QUEUES = frozenset(("tensor", "sync", "scalar", "vector"))
N_CORES = 8
B_FULL, D, C = 65536, 512, 10
B = B_FULL // N_CORES          # 8192 samples per core
N_QUBITS, Q_DEPTH = 4, 6
GROUPS = 16                    # groups of 512 samples
# units of n groups each (n col-strips of the PE); tapered so the last
# units' dependent chains are short
UNITS = [(0, 4), (4, 4), (8, 4), (12, 2), (14, 2)]

# f32 const blob column layout: pre_b | post_b | trigb | identity
FB_PREB = 0
FB_PB80 = 1
FB_TRIG = 2
FB_IDF = 4
FB_COLS = 4 + 128
# f32r const blob: mbd | pbd
RB_MBD = 0
RB_PBD = 128
RB_COLS = 128 + 80
# bf16 const blob: pre_wt | identity
BB_PWT = 0
BB_IDB = 128
BB_COLS = 256


# ---------------------------------------------------------------- host math
def _apply_1q(state, gate, wire):
    state = np.moveaxis(state, wire, 0)
    state = np.tensordot(gate, state, axes=((1,), (0,)))
    return np.moveaxis(state, 0, wire)


def _apply_cnot(state, ctrl, tgt):
    state = np.moveaxis(state, (ctrl, tgt), (0, 1))
    state = np.stack([state[0], state[1][::-1]], axis=0)
    return np.moveaxis(state, (0, 1), (ctrl, tgt))


def _ry(theta):
    c, s = np.cos(theta * 0.5), np.sin(theta * 0.5)
    return np.array([[c, -s], [s, c]])


def _build_M(q_params: np.ndarray) -> np.ndarray:
    """16x16 matrix of the fixed part of the circuit (after the per-sample
    RY layer): 6 repetitions of [CNOT(0,1), CNOT(2,3), CNOT(1,2), RY layer]."""
    qw = np.asarray(q_params, np.float64).reshape(Q_DEPTH, N_QUBITS)
    M = np.zeros((16, 16), np.float64)
    for i in range(16):
        state = np.zeros(16, np.float64)
        state[i] = 1.0
        state = state.reshape((2,) * N_QUBITS)
        for k in range(Q_DEPTH):
            for a in range(0, N_QUBITS - 1, 2):
                state = _apply_cnot(state, a, a + 1)
            for a in range(1, N_QUBITS - 1, 2):
                state = _apply_cnot(state, a, a + 1)
            for w in range(N_QUBITS):
                state = _apply_1q(state, _ry(qw[k, w]), w)
        M[:, i] = state.reshape(16)
    return M


def _build_P(post_w: np.ndarray) -> np.ndarray:
    """P[i, c] = sum_w post_w[c, w] * z_w(i), where z_w(i) flips sign with
    bit (3-w) of the state index i (axis 0 of the state = qubit 0)."""
    post_w = np.asarray(post_w, np.float64)
    i = np.arange(16)
    z = np.stack([1.0 - 2.0 * ((i >> (3 - w)) & 1) for w in range(N_QUBITS)], 1)
    return z @ post_w.T  # [16, 10]


# ---------------------------------------------------------------- bass build
def build_nc(sim_compat: bool = False) -> bass.Bass:
    # Bacc (not raw Bass): its finalize() runs generate_event_semaphores,
    # which splits multi-semaphore waits to satisfy the TRN2 one-wait-per-
    # instruction ISA limit.
    nc = bacc.Bacc(None)
    x = nc.dram_tensor("x", [B, D], BF16, kind="ExternalInput")
    fblob = nc.dram_tensor("fblob", [128, FB_COLS], F32, kind="ExternalInput")
    rblob = nc.dram_tensor("rblob", [128, RB_COLS], F32R, kind="ExternalInput")
    bblob = nc.dram_tensor("bblob", [128, BB_COLS], BF16, kind="ExternalInput")
    # transposed on device: out[(tile,comp) partition, 128*slab + p]
    out = nc.dram_tensor("out", [80, 1024], F32, kind="ExternalOutput")

    with ExitStack() as ctx:
        tc = ctx.enter_context(TileContext(nc))
        consts = ctx.enter_context(tc.tile_pool(name="consts", bufs=1))
        # all 16 xt group tiles stay resident (8 MB) — no WAR waits on the
        # transpose DMAs
        xt_pool = ctx.enter_context(tc.tile_pool(name="xt", bufs=GROUPS // 2))
        work = ctx.enter_context(tc.tile_pool(name="work", bufs=2))
        ps_po = ctx.enter_context(tc.tile_pool(name="ps_po", space="PSUM", bufs=2))
        ps_th = ctx.enter_context(tc.tile_pool(name="ps_th", space="PSUM", bufs=2))
        ps_ct = ctx.enter_context(tc.tile_pool(name="ps_ct", space="PSUM", bufs=1))

        last_on = {}

        def pin(engine_key, bass_ins):
            if engine_key not in PIN_QUEUES:
                return bass_ins
            prev = last_on.get(engine_key)
            if prev is not None:
                # add_dep_helper(a, b) = a depends on b: bass_ins AFTER prev
                add_dep_helper(bass_ins.ins, prev.ins, sync=False,
                               reason="queue order pin")
            last_on[engine_key] = bass_ins
            return bass_ins

        # ---- consts: 3 packed loads on the SCALAR queue (ACT ring).
        # Plain DMAs on the ACT ring concurrent with SP-ring xbar
        # transposes are safe (measured); keeping them off the sync queue
        # lets the first transpose start right after the preamble.
        fb_sb = consts.tile([128, FB_COLS], F32)
        pin("scalar", nc.scalar.dma_start(fb_sb, fblob[:, :]))
        rb_sb = consts.tile([128, RB_COLS], F32R)
        pin("scalar", nc.scalar.dma_start(rb_sb, rblob[:, :]))
        bb_sb = consts.tile([128, BB_COLS], BF16)
        pin("scalar", nc.scalar.dma_start(bb_sb, bblob[:, :]))
        pre_b_sb = fb_sb[:, FB_PREB:FB_PREB + 1]
        pb80_sb = fb_sb[0:80, FB_PB80:FB_PB80 + 1]
        trigb_sb = fb_sb[:, FB_TRIG:FB_TRIG + 2]
        idf_sb = fb_sb[:, FB_IDF:FB_IDF + 128]
        mbd_sb = rb_sb[:, RB_MBD:RB_MBD + 128]
        pbd_sb = rb_sb[:, RB_PBD:RB_PBD + 80]
        pre_wt_sb = bb_sb[:, BB_PWT:BB_PWT + 128]
        idb_sb = bb_sb[:, BB_IDB:BB_IDB + 128]

        out2_sb = consts.tile([80, 1024], F32)
        # tanh staging, bf16: [32n rows (32j+f), 512 cols per unit]
        tanh_sb = consts.tile([128, len(UNITS) * 512], BF16)

        # pin the activation table to silu_and_others once: it contains
        # silu+tanh+sin+square+identity, so no further table loads happen.
        # (CoreSim can't evaluate Silu; the sim build substitutes Tanh —
        # the value is unused either way.)
        silu_sb = consts.tile([128, 1], F32)
        pin("scalar", nc.scalar.activation(silu_sb, fb_sb[:, 0:1],
                                           AF.Tanh if sim_compat else AF.Silu))

        # ---- all 8 x transposes up-front on the sync queue (the
        # critical path); transpose i covers samples 1024i..1024(i+1)
        # (groups 2i, 2i+1).  8 big transposes instead of 16 keeps the
        # bacc event-semaphore pool from recycling mid-stream (recycling
        # waits were measured blocking the queue 4.4us twice).
        # NOTE: all xbar transposes must stay on ONE HWDGE queue —
        # concurrent transpose streams on the SP and ACT rings
        # corrupt data through the shared xbar (measured twice).
        xts = []
        for g in range(GROUPS // 2):
            xt = xt_pool.tile([128, 4 * 1024], BF16, name="xt", tag="xt")
            pin("sync", nc.sync.dma_start(
                xt[:, :].rearrange("p (k b) -> p k b", k=4),
                x[1024 * g:1024 * (g + 1), :],
                transpose=True))
            xts.append(xt)

        def group_rhs(g, k):
            # moving operand for group g, contraction chunk k
            return xts[g // 2][:, :].rearrange(
                "p (k b) -> p k b", k=4)[:, k, 512 * (g % 2):512 * (g % 2 + 1)]

        ocol = 0
        def unit_stages(u, g0, n, ocol):
            """Generator emitting one unit's pipeline; yields at stage
            boundaries so tail units can interleave their emission (each
            engine queue runs strictly in emission order)."""
            rows = 32 * n
            # ---- pre-net: n groups col-tiled onto PE strips 32j.
            # po[32j + f, s] = pre_out feature f of sample 512*(g0+j) + s.
            # psum pending-zero state is per-partition, so each col-strip
            # opens/closes its own accumulation group (start on its k=0,
            # stop on its k=3); skip_group_check silences the bank-granular
            # build-time checker which doesn't model per-strip groups.
            # Strip-major (j outer) order: strip j's 4-matmul chain starts
            # as soon as ITS group's transpose lands, and neighboring
            # strips' chains overlap on the PE (distinct col-groups).
            po = ps_po.tile([128, 512], F32, name="po", tag="po")
            for j in range(n):
                for k in range(4):
                    pin("tensor", nc.tensor.matmul(
                        po[32 * j:32 * (j + 1), :],
                        lhsT=pre_wt_sb[:, 32 * k:32 * k + 32],
                        rhs=group_rhs(g0 + j, k),
                        start=(k == 0), stop=(k == 3),
                        tile_position=(0, 32 * j),
                        skip_group_check=True))
            yield "pre"
            # fused bias + tanh on the whole unit, bf16 out
            tq = tanh_sb[0:rows, 512 * u:512 * (u + 1)]
            pin("scalar", nc.scalar.activation(tq, po[0:rows, :], AF.Tanh,
                                               bias=fb_sb[0:rows,
                                                          FB_PREB:FB_PREB + 1]))
            yield "tanh"

            # ---- flip to sample-major: 4 PE transposes [32n,128] -> psum.
            # thT[p, 32n*k + 32j + f] = tanh feature f of sample
            # 512*(g0+j) + 128k + p  (cols 32j+4..32j+31 are garbage)
            thT = ps_th.tile([128, 512], BF16, name="thT", tag="thT")
            for k in range(4):
                pin("tensor", nc.tensor.transpose(
                    thT[:, rows * k:rows * (k + 1)],
                    tq[:, 128 * k:128 * (k + 1)], idb_sb[0:rows, 0:rows]))
            yield "th"

            # ---- trig: cos/sin of theta = PI4*t + {3pi/4, pi/4}
            # cs[p, (k, j, w, x)]
            cs = work.tile([128, 128], F32, name="cs", tag="cs")
            cs5 = cs[:, 0:32 * n].rearrange("p (k j w x) -> p k j w x",
                                            k=4, j=n, w=4, x=2)
            thT4 = thT[:, 0:4 * rows].rearrange("p (k j w) -> p k j w",
                                                k=4, j=n)
            pin("scalar", nc.scalar.activation(
                cs5[:, :, :, :, 0], thT4[:, :, :, 0:4],
                AF.Sin, bias=trigb_sb[:, 0:1], scale=PI4))
            pin("scalar", nc.scalar.activation(
                cs5[:, :, :, :, 1], thT4[:, :, :, 0:4],
                AF.Sin, bias=trigb_sb[:, 1:2], scale=PI4))
            yield "trig"

            # ---- psi = v0 (x) v1 (x) v2 (x) v3 per tile kj (kj = k*n+j)
            nt = 4 * n   # sample tiles in this unit
            cs4 = cs[:, 0:32 * n].rearrange("p (kj w x) -> p kj w x",
                                            w=4, x=2)
            v01 = work.tile([128, 64], F32, name="v01", tag="v01")
            v23 = work.tile([128, 64], F32, name="v23", tag="v23")
            pin("vector", nc.vector.tensor_tensor(
                out=v01[:, 0:4 * nt].rearrange("p (t a b) -> p t a b",
                                               a=2, b=2),
                in0=cs4[:, :, 0, :].unsqueeze(3).broadcast_to((128, nt, 2, 2)),
                in1=cs4[:, :, 1, :].unsqueeze(2).broadcast_to((128, nt, 2, 2)),
                op=mybir.AluOpType.mult))
            pin("vector", nc.vector.tensor_tensor(
                out=v23[:, 0:4 * nt].rearrange("p (t a b) -> p t a b",
                                               a=2, b=2),
                in0=cs4[:, :, 2, :].unsqueeze(3).broadcast_to((128, nt, 2, 2)),
                in1=cs4[:, :, 3, :].unsqueeze(2).broadcast_to((128, nt, 2, 2)),
                op=mybir.AluOpType.mult))
            psi = work.tile([128, 256], F32, name="psi", tag="psi")
            pin("vector", nc.vector.tensor_tensor(
                out=psi[:, 0:16 * nt].rearrange("p (t a b) -> p t a b",
                                                a=4, b=4),
                in0=v01[:, 0:4 * nt].rearrange("p (t i) -> p t i", i=4)
                    .unsqueeze(3).broadcast_to((128, nt, 4, 4)),
                in1=v23[:, 0:4 * nt].rearrange("p (t i) -> p t i", i=4)
                    .unsqueeze(2).broadcast_to((128, nt, 4, 4)),
                op=mybir.AluOpType.mult))
            yield "dve"

            # ---- quantum circuit per slab of 8 tiles (nt/8 slabs), all
            # slabs of the unit share one 128*ns-col M and P matmul
            # (float32r: full-rate rows at N>=256, ~fp32 accuracy)
            ns = nt // 8
            psiT = work.tile([128, 256], F32R, name="psiT", tag="psiT")
            for h in range(ns):
                psiT_ps = ps_ct.tile([128, 128], F32, name="psiT_ps", tag="pT")
                pin("tensor", nc.tensor.transpose(
                    psiT_ps, psi[:, 128 * h:128 * (h + 1)], idf_sb))
                pin("vector", nc.vector.tensor_copy(
                    psiT[:, 128 * h:128 * (h + 1)], psiT_ps))
            phiT_ps = ps_ct.tile([128, 256], F32, name="phiT_ps", tag="phT")
            pin("tensor", nc.tensor.matmul(
                phiT_ps[:, 0:128 * ns], lhsT=mbd_sb, rhs=psiT[:, 0:128 * ns],
                start=True, stop=True))
            phi2 = work.tile([128, 256], F32R, name="phi2", tag="phi2")
            pin("scalar", nc.scalar.activation(
                phi2[:, 0:128 * ns], phiT_ps[:, 0:128 * ns], AF.Square))
            o10_ps = ps_ct.tile([80, 256], F32, name="o10_ps", tag="o10")
            pin("tensor", nc.tensor.matmul(
                o10_ps[:, 0:128 * ns], lhsT=pbd_sb, rhs=phi2[:, 0:128 * ns],
                start=True, stop=True))
            # bias-add into the output staging tile, then store this unit's
            # slice right away (plain copy on the ACT ring; overlaps the
            # SP-ring transpose stream)
            pin("scalar", nc.scalar.activation(
                out2_sb[:, ocol:ocol + 128 * ns],
                o10_ps[:, 0:128 * ns], AF.Identity, bias=pb80_sb))
            pin("scalar", nc.scalar.dma_start(
                out[:, ocol:ocol + 128 * ns],
                out2_sb[:, ocol:ocol + 128 * ns]))
            yield "store"

        # ---- drive the units: 0..N-3 sequential; the last two interleave
        # so their dependent chains overlap after the final transposes.
        gens = []
        col = 0
        for u, (g0, n) in enumerate(UNITS):
            gens.append(unit_stages(u, g0, n, col))
            col += 128 * ((4 * n) // 8)

        def run_all(gen):
            for _ in gen:
                pass

        def run_until(gen, stage):
            for s in gen:
                if s == stage:
                    return

        for g in gens[:-2]:
            run_all(g)
        ga, gb = gens[-2], gens[-1]
        run_until(ga, "trig")   # pre, tanh, th, trig emitted
        run_until(gb, "tanh")   # pre, tanh emitted
        run_all(ga)
        run_all(gb)

    nc.finalize()  # bacc: register alloc + event-semaphore wait splitting
    return nc


_NC_CACHE: dict = {}


def _get_nc() -> bass.Bass:
    if "nc" not in _NC_CACHE:
        _NC_CACHE["nc"] = build_nc()
    return _NC_CACHE["nc"]


def make_in_maps(inputs: dict) -> list:
    x = np.asarray(inputs["input_features"], np.float32)
    pre_w = np.asarray(inputs["pre_w"], np.float32)
    pre_b = np.asarray(inputs["pre_b"], np.float32)
    q_params = np.asarray(inputs["q_params"], np.float32)
    post_w = np.asarray(inputs["post_w"], np.float32)
    post_b = np.asarray(inputs["post_b"], np.float32)

    M = _build_M(q_params)
    P = _build_P(post_w)
    rblob = np.zeros((128, RB_COLS), np.float32)
    for t in range(8):
        rblob[16 * t:16 * (t + 1), RB_MBD + 16 * t:RB_MBD + 16 * (t + 1)] = M.T
        rblob[16 * t:16 * (t + 1), RB_PBD + 10 * t:RB_PBD + 10 * (t + 1)] = P

    fblob = np.zeros((128, FB_COLS), np.float32)
    for j in range(4):
        fblob[32 * j:32 * j + 4, FB_PREB] = pre_b
    fblob[0:80, FB_PB80] = np.tile(post_b, 8)
    fblob[:, FB_TRIG + 0] = 3.0 * PI4
    fblob[:, FB_TRIG + 1] = PI4
    fblob[:, FB_IDF:FB_IDF + 128] = np.eye(128, dtype=np.float32)

    bblob = np.zeros((128, BB_COLS), np.float32)
    # pre_wt[p, 32k + f] = pre_w[f, 128k+p], zero-padded to 32 cols/chunk
    for k in range(4):
        bblob[:, BB_PWT + 32 * k:BB_PWT + 32 * k + 4] = \
            pre_w[:, 128 * k:128 * (k + 1)].T
    bblob[:, BB_IDB:BB_IDB + 128] = np.eye(128, dtype=np.float32)
    bblob = bblob.astype(ml_dtypes.bfloat16)

    xb = x.astype(ml_dtypes.bfloat16)
    consts = dict(fblob=fblob, rblob=rblob, bblob=bblob)
    return [dict(x=xb[B * i:B * (i + 1)], **consts) for i in range(N_CORES)]


def _out_index() -> np.ndarray:
    """Map device out [80, 1024] -> sample/class gather indices.

    Device col 128*s + p (s = global slab) and partition 10*m + c hold
    class c of sample 512*(g0+j) + 128k + p, where within the slab's
    unit kj = 8*h + m (h = slab index within the unit), k = kj//n,
    j = kj%n.
    """
    idx_p = np.zeros((B, C), np.int64)
    idx_c = np.zeros((B, C), np.int64)
    s = 0
    for (g0, n) in UNITS:
        for h in range(n // 2):
            for m in range(8):
                kj = 8 * h + m
                k, j = kj // n, kj % n
                base = 512 * (g0 + j) + 128 * k
                samples = base + np.arange(128)
                for c in range(C):
                    idx_p[samples, c] = 10 * m + c
                    idx_c[samples, c] = 128 * s + np.arange(128)
            s += 1
    return np.stack([idx_p, idx_c], axis=-1)


_OUT_IDX = _out_index()


def unpack_out(dev_out: np.ndarray) -> np.ndarray:
    """[80, 1024] device layout -> [B, C]."""
    return np.ascontiguousarray(dev_out[_OUT_IDX[..., 0], _OUT_IDX[..., 1]])


def run_on_device(inputs: dict, **kwargs):
    """Returns (full_output, BassKernelResults)."""
    nc = _get_nc()
    in_maps = make_in_maps(inputs)
    res = run_bass_kernel_spmd(nc, in_maps, core_ids=list(range(N_CORES)),
                               **kwargs)
    full = np.concatenate(
        [unpack_out(res.results[i]["out"]) for i in range(N_CORES)], 0)
    return np.ascontiguousarray(full, dtype=np.float32), res


def kernel(**inputs) -> np.ndarray:
    out, _ = run_on_device(inputs)
    return out


# revision 23
# speedup vs baseline: 1.2086x; 1.2086x over previous
"""Trainium2 Bass kernel for nn_DressedQuantumNet.

Math reformulation (exact, up to float rounding):
  pre_out = x @ pre_w.T + pre_b                  # [B,4]
  theta_w = (pi/4)*tanh(pre_out_w) + pi/4        # in (0, pi/2)
  v_w     = [cos theta_w, sin theta_w]           # per-qubit state (positive)
  psi     = v_0 (x) v_1 (x) v_2 (x) v_3          # [B,16] product state
  phi     = M @ psi        # M = fixed 16x16 matrix of the CNOT/RY circuit
  out     = (phi*phi)^T P + post_b  # P[i,c] = sum_w post_w[c,w] * z_w(i)

Device strategy (pure data parallel over 8 cores, 8192 samples each):
  - x bf16, loaded transposed via the DMA xbar on the sync queue; the 16
    group transposes are the critical path, so the sync queue carries
    only 3 packed const loads followed by the 16 transposes.
  - pre-matmul is PE col-tiled: the n groups of a unit go to col-strips
    32j of the PE array (tile_position), so their 512-col matmuls run
    concurrently and the psum output is a dense [32n, 512] tile.
  - units are tapered [4,4,4,2,2] groups so the last unit's dependent
    chain (the kernel tail after the final transpose) is short.
  - tanh is one [32n,512] activation per unit; the [feature, sample] ->
    [sample, feature] flip is 4 PE transposes per unit (bf16 -> psum);
    no SBUF->SBUF xbar transposes, no memset.
  - trig on ScalarE reads the transposed psum directly (2x Sin with
    scale/bias folding cos); psi built with 3 broadcast-AP vector mults.
  - quantum circuit: PE transpose of psi -> [(tile,comp), sample], then
    block-diagonal M (16x16 x8) and P (16x10 x8) matmuls in float32r
    (full-rate rows, ~fp32 accuracy).
  - per-unit output stores on the scalar queue overlap the x stream.
  - every engine queue is pinned to emission order (sync=False deps):
    queues execute in-order at runtime, and the Tile scheduler's DMA
    model otherwise reorders them badly around the long transposes.
"""

import os
import sys

for _p in ("/opt/trn_rl_repo",):
    if os.path.isdir(_p) and _p not in sys.path:
        sys.path.insert(0, _p)

import math
import numpy as np
import ml_dtypes
from contextlib import ExitStack

import concourse.bass as bass
import concourse.bacc as bacc
import concourse.mybir as mybir
from concourse.tile import TileContext, add_dep_helper
from concourse.bass_utils import run_bass_kernel_spmd

F32 = mybir.dt.float32
F32R = mybir.dt.float32r
BF16 = mybir.dt.bfloat16
AF = mybir.ActivationFunctionType
PI4 = math.pi / 4.0

PIN_QUEUES = frozenset(("tensor", "sync", "scalar", "vector"))
N_CORES = 8
B_FULL, D, C = 65536, 512, 10
B = B_FULL // N_CORES          # 8192 samples per core
N_QUBITS, Q_DEPTH = 4, 6
GROUPS = 16                    # groups of 512 samples
# units of n groups each (n col-strips of the PE); tapered so the last
# units' dependent chains are short
UNITS = [(0, 4), (4, 4), (8, 4), (12, 2), (14, 2)]

# f32 const blob column layout: pre_b | post_b | trigb | identity
FB_PREB = 0
FB_PB80 = 1
FB_TRIG = 2
FB_IDF = 4
FB_COLS = 4 + 128
# f32r const blob: mbd | pbd
RB_MBD = 0
RB_PBD = 128
RB_COLS = 128 + 80
# bf16 const blob: pre_wt | identity
BB_PWT = 0
BB_IDB = 128
BB_COLS = 256


# ---------------------------------------------------------------- host math
def _apply_1q(state, gate, wire):
    state = np.moveaxis(state, wire, 0)
    state = np.tensordot(gate, state, axes=((1,), (0,)))
    return np.moveaxis(state, 0, wire)


def _apply_cnot(state, ctrl, tgt):
    state = np.moveaxis(state, (ctrl, tgt), (0, 1))
    state = np.stack([state[0], state[1][::-1]], axis=0)
    return np.moveaxis(state, (0, 1), (ctrl, tgt))


def _ry(theta):
    c, s = np.cos(theta * 0.5), np.sin(theta * 0.5)
    return np.array([[c, -s], [s, c]])


def _build_M(q_params: np.ndarray) -> np.ndarray:
    """16x16 matrix of the fixed part of the circuit (after the per-sample
    RY layer): 6 repetitions of [CNOT(0,1), CNOT(2,3), CNOT(1,2), RY layer]."""
    qw = np.asarray(q_params, np.float64).reshape(Q_DEPTH, N_QUBITS)
    M = np.zeros((16, 16), np.float64)
    for i in range(16):
        state = np.zeros(16, np.float64)
        state[i] = 1.0
        state = state.reshape((2,) * N_QUBITS)
        for k in range(Q_DEPTH):
            for a in range(0, N_QUBITS - 1, 2):
                state = _apply_cnot(state, a, a + 1)
            for a in range(1, N_QUBITS - 1, 2):
                state = _apply_cnot(state, a, a + 1)
            for w in range(N_QUBITS):
                state = _apply_1q(state, _ry(qw[k, w]), w)
        M[:, i] = state.reshape(16)
    return M


def _build_P(post_w: np.ndarray) -> np.ndarray:
    """P[i, c] = sum_w post_w[c, w] * z_w(i), where z_w(i) flips sign with
    bit (3-w) of the state index i (axis 0 of the state = qubit 0)."""
    post_w = np.asarray(post_w, np.float64)
    i = np.arange(16)
    z = np.stack([1.0 - 2.0 * ((i >> (3 - w)) & 1) for w in range(N_QUBITS)], 1)
    return z @ post_w.T  # [16, 10]


# ---------------------------------------------------------------- bass build
def build_nc(sim_compat: bool = False) -> bass.Bass:
    # Bacc (not raw Bass): its finalize() runs generate_event_semaphores,
    # which splits multi-semaphore waits to satisfy the TRN2 one-wait-per-
    # instruction ISA limit.
    nc = bacc.Bacc(None)
    x = nc.dram_tensor("x", [B, D], BF16, kind="ExternalInput")
    fblob = nc.dram_tensor("fblob", [128, FB_COLS], F32, kind="ExternalInput")
    rblob = nc.dram_tensor("rblob", [128, RB_COLS], F32R, kind="ExternalInput")
    bblob = nc.dram_tensor("bblob", [128, BB_COLS], BF16, kind="ExternalInput")
    # transposed on device: out[(tile,comp) partition, 128*slab + p]
    out = nc.dram_tensor("out", [80, 1024], F32, kind="ExternalOutput")

    with ExitStack() as ctx:
        tc = ctx.enter_context(TileContext(nc))
        consts = ctx.enter_context(tc.tile_pool(name="consts", bufs=1))
        # all 16 xt group tiles stay resident (8 MB) — no WAR waits on the
        # transpose DMAs
        xt_pool = ctx.enter_context(tc.tile_pool(name="xt", bufs=GROUPS // 2))
        work = ctx.enter_context(tc.tile_pool(name="work", bufs=2))
        ps_po = ctx.enter_context(tc.tile_pool(name="ps_po", space="PSUM", bufs=2))
        ps_th = ctx.enter_context(tc.tile_pool(name="ps_th", space="PSUM", bufs=2))
        ps_ct = ctx.enter_context(tc.tile_pool(name="ps_ct", space="PSUM", bufs=1))

        last_on = {}

        def pin(engine_key, bass_ins):
            if engine_key not in PIN_QUEUES:
                return bass_ins
            prev = last_on.get(engine_key)
            if prev is not None:
                # add_dep_helper(a, b) = a depends on b: bass_ins AFTER prev
                add_dep_helper(bass_ins.ins, prev.ins, sync=False,
                               reason="queue order pin")
            last_on[engine_key] = bass_ins
            return bass_ins

        # ---- consts: 3 packed loads on the SCALAR queue (ACT ring).
        # Plain DMAs on the ACT ring concurrent with SP-ring xbar
        # transposes are safe (measured); keeping them off the sync queue
        # lets the first transpose start right after the preamble.
        fb_sb = consts.tile([128, FB_COLS], F32)
        pin("scalar", nc.scalar.dma_start(fb_sb, fblob[:, :]))
        rb_sb = consts.tile([128, RB_COLS], F32R)
        pin("scalar", nc.scalar.dma_start(rb_sb, rblob[:, :]))
        bb_sb = consts.tile([128, BB_COLS], BF16)
        last_const = pin("scalar", nc.scalar.dma_start(bb_sb, bblob[:, :]))
        pre_b_sb = fb_sb[:, FB_PREB:FB_PREB + 1]
        pb80_sb = fb_sb[0:80, FB_PB80:FB_PB80 + 1]
        trigb_sb = fb_sb[:, FB_TRIG:FB_TRIG + 2]
        idf_sb = fb_sb[:, FB_IDF:FB_IDF + 128]
        mbd_sb = rb_sb[:, RB_MBD:RB_MBD + 128]
        pbd_sb = rb_sb[:, RB_PBD:RB_PBD + 80]
        pre_wt_sb = bb_sb[:, BB_PWT:BB_PWT + 128]
        idb_sb = bb_sb[:, BB_IDB:BB_IDB + 128]

        out2_sb = consts.tile([80, 1024], F32)
        # tanh staging, bf16: [32n rows (32j+f), 512 cols per unit]
        tanh_sb = consts.tile([128, len(UNITS) * 512], BF16)

        # pin the activation table to silu_and_others once: it contains
        # silu+tanh+sin+square+identity, so no further table loads happen.
        # (CoreSim can't evaluate Silu; the sim build substitutes Tanh —
        # the value is unused either way.)
        silu_sb = consts.tile([128, 1], F32)
        pin("scalar", nc.scalar.activation(silu_sb, fb_sb[:, 0:1],
                                           AF.Tanh if sim_compat else AF.Silu))

        # ---- all 8 x transposes up-front on the sync queue (the
        # critical path); transpose i covers samples 1024i..1024(i+1)
        # (groups 2i, 2i+1).  8 big transposes instead of 16 keeps the
        # bacc event-semaphore pool from recycling mid-stream (recycling
        # waits were measured blocking the queue 4.4us twice).
        # NOTE: all xbar transposes must stay on ONE HWDGE queue —
        # concurrent transpose streams on the SP and ACT rings
        # corrupt data through the shared xbar (measured twice).
        xts = []
        for g in range(GROUPS // 2):
            xt = xt_pool.tile([128, 4 * 1024], BF16, name="xt", tag="xt")
            xpose = pin("sync", nc.sync.dma_start(
                xt[:, :].rearrange("p (k b) -> p k b", k=4),
                x[1024 * g:1024 * (g + 1), :],
                transpose=True))
            # the scheduler serializes every copy<->transpose transition in
            # its global DMA order with cross-queue semaphore chains; force
            # all plain const copies before every transpose so the chain
            # has a single transition
            add_dep_helper(xpose.ins, last_const.ins, sync=False,
                           reason="consts before xbar transposes")
            xts.append(xt)

        def group_rhs(g, k):
            # moving operand for group g, contraction chunk k
            return xts[g // 2][:, :].rearrange(
                "p (k b) -> p k b", k=4)[:, k, 512 * (g % 2):512 * (g % 2 + 1)]

        ocol = 0
        def unit_stages(u, g0, n, ocol):
            """Generator emitting one unit's pipeline; yields at stage
            boundaries so tail units can interleave their emission (each
            engine queue runs strictly in emission order)."""
            rows = 32 * n
            # ---- pre-net: n groups col-tiled onto PE strips 32j.
            # po[32j + f, s] = pre_out feature f of sample 512*(g0+j) + s.
            # psum pending-zero state is per-partition, so each col-strip
            # opens/closes its own accumulation group (start on its k=0,
            # stop on its k=3); skip_group_check silences the bank-granular
            # build-time checker which doesn't model per-strip groups.
            # Strip-major (j outer) order: strip j's 4-matmul chain starts
            # as soon as ITS group's transpose lands, and neighboring
            # strips' chains overlap on the PE (distinct col-groups).
            po = ps_po.tile([128, 512], F32, name="po", tag="po")
            for j in range(n):
                for k in range(4):
                    pin("tensor", nc.tensor.matmul(
                        po[32 * j:32 * (j + 1), :],
                        lhsT=pre_wt_sb[:, 32 * k:32 * k + 32],
                        rhs=group_rhs(g0 + j, k),
                        start=(k == 0), stop=(k == 3),
                        tile_position=(0, 32 * j),
                        skip_group_check=True))
            yield "pre"
            # fused bias + tanh on the whole unit, bf16 out
            tq = tanh_sb[0:rows, 512 * u:512 * (u + 1)]
            pin("scalar", nc.scalar.activation(tq, po[0:rows, :], AF.Tanh,
                                               bias=fb_sb[0:rows,
                                                          FB_PREB:FB_PREB + 1]))
            yield "tanh"

            # ---- flip to sample-major: 4 PE transposes [32n,128] -> psum.
            # thT[p, 32n*k + 32j + f] = tanh feature f of sample
            # 512*(g0+j) + 128k + p  (cols 32j+4..32j+31 are garbage)
            thT = ps_th.tile([128, 512], BF16, name="thT", tag="thT")
            for k in range(4):
                pin("tensor", nc.tensor.transpose(
                    thT[:, rows * k:rows * (k + 1)],
                    tq[:, 128 * k:128 * (k + 1)], idb_sb[0:rows, 0:rows]))
            yield "th"

            # ---- trig: cos/sin of theta = PI4*t + {3pi/4, pi/4}
            # cs[p, (k, j, w, x)]
            cs = work.tile([128, 128], F32, name="cs", tag="cs")
            cs5 = cs[:, 0:32 * n].rearrange("p (k j w x) -> p k j w x",
                                            k=4, j=n, w=4, x=2)
            thT4 = thT[:, 0:4 * rows].rearrange("p (k j w) -> p k j w",
                                                k=4, j=n)
            pin("scalar", nc.scalar.activation(
                cs5[:, :, :, :, 0], thT4[:, :, :, 0:4],
                AF.Sin, bias=trigb_sb[:, 0:1], scale=PI4))
            pin("scalar", nc.scalar.activation(
                cs5[:, :, :, :, 1], thT4[:, :, :, 0:4],
                AF.Sin, bias=trigb_sb[:, 1:2], scale=PI4))
            yield "trig"

            # ---- psi = v0 (x) v1 (x) v2 (x) v3 per tile kj (kj = k*n+j)
            nt = 4 * n   # sample tiles in this unit
            cs4 = cs[:, 0:32 * n].rearrange("p (kj w x) -> p kj w x",
                                            w=4, x=2)
            v01 = work.tile([128, 64], F32, name="v01", tag="v01")
            v23 = work.tile([128, 64], F32, name="v23", tag="v23")
            pin("vector", nc.vector.tensor_tensor(
                out=v01[:, 0:4 * nt].rearrange("p (t a b) -> p t a b",
                                               a=2, b=2),
                in0=cs4[:, :, 0, :].unsqueeze(3).broadcast_to((128, nt, 2, 2)),
                in1=cs4[:, :, 1, :].unsqueeze(2).broadcast_to((128, nt, 2, 2)),
                op=mybir.AluOpType.mult))
            pin("vector", nc.vector.tensor_tensor(
                out=v23[:, 0:4 * nt].rearrange("p (t a b) -> p t a b",
                                               a=2, b=2),
                in0=cs4[:, :, 2, :].unsqueeze(3).broadcast_to((128, nt, 2, 2)),
                in1=cs4[:, :, 3, :].unsqueeze(2).broadcast_to((128, nt, 2, 2)),
                op=mybir.AluOpType.mult))
            psi = work.tile([128, 256], F32, name="psi", tag="psi")
            pin("vector", nc.vector.tensor_tensor(
                out=psi[:, 0:16 * nt].rearrange("p (t a b) -> p t a b",
                                                a=4, b=4),
                in0=v01[:, 0:4 * nt].rearrange("p (t i) -> p t i", i=4)
                    .unsqueeze(3).broadcast_to((128, nt, 4, 4)),
                in1=v23[:, 0:4 * nt].rearrange("p (t i) -> p t i", i=4)
                    .unsqueeze(2).broadcast_to((128, nt, 4, 4)),
                op=mybir.AluOpType.mult))
            yield "dve"

            # ---- quantum circuit per slab of 8 tiles (nt/8 slabs), all
            # slabs of the unit share one 128*ns-col M and P matmul
            # (float32r: full-rate rows at N>=256, ~fp32 accuracy)
            ns = nt // 8
            psiT = work.tile([128, 256], F32R, name="psiT", tag="psiT")
            for h in range(ns):
                psiT_ps = ps_ct.tile([128, 128], F32, name="psiT_ps", tag="pT")
                pin("tensor", nc.tensor.transpose(
                    psiT_ps, psi[:, 128 * h:128 * (h + 1)], idf_sb))
                pin("vector", nc.vector.tensor_copy(
                    psiT[:, 128 * h:128 * (h + 1)], psiT_ps))
            phiT_ps = ps_ct.tile([128, 256], F32, name="phiT_ps", tag="phT")
            pin("tensor", nc.tensor.matmul(
                phiT_ps[:, 0:128 * ns], lhsT=mbd_sb, rhs=psiT[:, 0:128 * ns],
                start=True, stop=True))
            phi2 = work.tile([128, 256], F32R, name="phi2", tag="phi2")
            pin("scalar", nc.scalar.activation(
                phi2[:, 0:128 * ns], phiT_ps[:, 0:128 * ns], AF.Square))
            o10_ps = ps_ct.tile([80, 256], F32, name="o10_ps", tag="o10")
            pin("tensor", nc.tensor.matmul(
                o10_ps[:, 0:128 * ns], lhsT=pbd_sb, rhs=phi2[:, 0:128 * ns],
                start=True, stop=True))
            # bias-add into the output staging tile (stored once at the
            # very end: a mid-stream store DMA would join the scheduler's
            # copy<->transpose serialization chain and stall the stream)
            pin("scalar", nc.scalar.activation(
                out2_sb[:, ocol:ocol + 128 * ns],
                o10_ps[:, 0:128 * ns], AF.Identity, bias=pb80_sb))
            yield "store"

        # ---- drive the units: 0..N-3 sequential; the last two interleave
        # so their dependent chains overlap after the final transposes.
        gens = []
        col = 0
        for u, (g0, n) in enumerate(UNITS):
            gens.append(unit_stages(u, g0, n, col))
            col += 128 * ((4 * n) // 8)

        def run_all(gen):
            for _ in gen:
                pass

        def run_until(gen, stage):
            for s in gen:
                if s == stage:
                    return

        for g in gens[:-2]:
            run_all(g)
        ga, gb = gens[-2], gens[-1]
        run_until(ga, "trig")   # pre, tanh, th, trig emitted
        run_until(gb, "tanh")   # pre, tanh emitted
        run_all(ga)
        run_all(gb)

        # single output store at the very end
        pin("scalar", nc.scalar.dma_start(out[:, :], out2_sb[:, :]))

    nc.finalize()  # bacc: register alloc + event-semaphore wait splitting
    return nc


_NC_CACHE: dict = {}


def _get_nc() -> bass.Bass:
    if "nc" not in _NC_CACHE:
        _NC_CACHE["nc"] = build_nc()
    return _NC_CACHE["nc"]


def make_in_maps(inputs: dict) -> list:
    x = np.asarray(inputs["input_features"], np.float32)
    pre_w = np.asarray(inputs["pre_w"], np.float32)
    pre_b = np.asarray(inputs["pre_b"], np.float32)
    q_params = np.asarray(inputs["q_params"], np.float32)
    post_w = np.asarray(inputs["post_w"], np.float32)
    post_b = np.asarray(inputs["post_b"], np.float32)

    M = _build_M(q_params)
    P = _build_P(post_w)
    rblob = np.zeros((128, RB_COLS), np.float32)
    for t in range(8):
        rblob[16 * t:16 * (t + 1), RB_MBD + 16 * t:RB_MBD + 16 * (t + 1)] = M.T
        rblob[16 * t:16 * (t + 1), RB_PBD + 10 * t:RB_PBD + 10 * (t + 1)] = P

    fblob = np.zeros((128, FB_COLS), np.float32)
    for j in range(4):
        fblob[32 * j:32 * j + 4, FB_PREB] = pre_b
    fblob[0:80, FB_PB80] = np.tile(post_b, 8)
    fblob[:, FB_TRIG + 0] = 3.0 * PI4
    fblob[:, FB_TRIG + 1] = PI4
    fblob[:, FB_IDF:FB_IDF + 128] = np.eye(128, dtype=np.float32)

    bblob = np.zeros((128, BB_COLS), np.float32)
    # pre_wt[p, 32k + f] = pre_w[f, 128k+p], zero-padded to 32 cols/chunk
    for k in range(4):
        bblob[:, BB_PWT + 32 * k:BB_PWT + 32 * k + 4] = \
            pre_w[:, 128 * k:128 * (k + 1)].T
    bblob[:, BB_IDB:BB_IDB + 128] = np.eye(128, dtype=np.float32)
    bblob = bblob.astype(ml_dtypes.bfloat16)

    xb = x.astype(ml_dtypes.bfloat16)
    consts = dict(fblob=fblob, rblob=rblob, bblob=bblob)
    return [dict(x=xb[B * i:B * (i + 1)], **consts) for i in range(N_CORES)]


def _out_index() -> np.ndarray:
    """Map device out [80, 1024] -> sample/class gather indices.

    Device col 128*s + p (s = global slab) and partition 10*m + c hold
    class c of sample 512*(g0+j) + 128k + p, where within the slab's
    unit kj = 8*h + m (h = slab index within the unit), k = kj//n,
    j = kj%n.
    """
    idx_p = np.zeros((B, C), np.int64)
    idx_c = np.zeros((B, C), np.int64)
    s = 0
    for (g0, n) in UNITS:
        for h in range(n // 2):
            for m in range(8):
                kj = 8 * h + m
                k, j = kj // n, kj % n
                base = 512 * (g0 + j) + 128 * k
                samples = base + np.arange(128)
                for c in range(C):
                    idx_p[samples, c] = 10 * m + c
                    idx_c[samples, c] = 128 * s + np.arange(128)
            s += 1
    return np.stack([idx_p, idx_c], axis=-1)


_OUT_IDX = _out_index()


def unpack_out(dev_out: np.ndarray) -> np.ndarray:
    """[80, 1024] device layout -> [B, C]."""
    return np.ascontiguousarray(dev_out[_OUT_IDX[..., 0], _OUT_IDX[..., 1]])


def run_on_device(inputs: dict, **kwargs):
    """Returns (full_output, BassKernelResults)."""
    nc = _get_nc()
    in_maps = make_in_maps(inputs)
    res = run_bass_kernel_spmd(nc, in_maps, core_ids=list(range(N_CORES)),
                               **kwargs)
    full = np.concatenate(
        [unpack_out(res.results[i]["out"]) for i in range(N_CORES)], 0)
    return np.ascontiguousarray(full, dtype=np.float32), res


def kernel(**inputs) -> np.ndarray:
    out, _ = run_on_device(inputs)
    return out
